# revision 27
# baseline (speedup 1.0000x reference)
"""Trainium2 Bass kernel for nn_CrossAttentionModule (head-collapsed cross attention).

Math (reference):
    Q = x @ Wq.T ; K = y @ Wk.T ; V = y @ Wv.T          (torch Linear convention)
    energy[n,q,k] = sum_{h,d} Q[n,q,h,d] K[n,k,h,d]     (heads summed!)
    att = softmax(energy / sqrt(512), axis=k)
    out = x + (att @ V) @ Wo.T + bo

Because heads are summed, energy = x @ (Wq.T @ Wk) @ y.T and the output
projection folds into V:  (att @ V) @ Wo.T = att @ (y @ (Wo @ Wv).T).
So we precompute on host (512x512, trivial):
    A    = Wq.T @ Wk        -> energy = (x @ A) @ y.T
    WvoT = Wv.T @ Wo.T      -> Vp = y @ WvoT ; att_out = att @ Vp
Device (per core, data-parallel over the N=8 batch):
    tT = A.T @ xT           [e2, q]   bf16
    Vp = y @ WvoT           [k, f]    bf16
    S^T tiles  = yT.T @ tT  [k, q]    fp32 psum   (k on partitions)
    P = exp(S^T * 1/sqrt(512))        bf16
    att_psum  += P.T @ Vp   [q, f]    fp32 psum   (accumulated over k tiles)
    den_psum  += P.T @ ones [q, 1]    fp32 psum
    out = att_psum * (1/den)          fp32 -> DRAM
Host adds the residual x + out + bo in fp32.
"""

import sys

sys.path.insert(0, "/opt/trn_rl_repo")

import ml_dtypes
import numpy as np

import bass_rust
import concourse.bass as bass
import concourse.mybir as mybir
import concourse.tile as tile
from concourse.bass_utils import run_bass_kernel_spmd
from concourse.vector_clock import ScopedClock

N_CORES = 8
E = 512  # embed dim
Q = 2048  # query length (per batch element)
K = 4096  # key/value length
P = 128  # partitions
ET = E // P  # 4 embed tiles
QB = 512  # q block width for S^T matmuls
NQB = Q // QB  # 4
QS = P  # q sub-block (att psum partition dim)
NQS = QB // QS  # 4
KT = K // P  # 32 k tiles
SCALE = float(1.0 / np.sqrt(np.float32(512.0)))

BF16 = mybir.dt.bfloat16
F32 = mybir.dt.float32
FP8E4 = mybir.dt.float8e4
FP8E5 = mybir.dt.float8e5
BF16_NP = ml_dtypes.bfloat16
E4_NP = ml_dtypes.float8_e4m3
E5_NP = ml_dtypes.float8_e5m2

# fp8 DoubleRow for the S^T / att / den / Vp matmuls (2x PE throughput on the
# dominant GEMMs). exp outputs use e5m2: P values span [3e-4, 3.3e3], which
# fits e5m2's exponent range with no shift; e4m3 would clip the tail.
USE_FP8 = True


def _patched_drain_and_barrier(self, tick_clock, wait_clock):
    # The walrus build in this container caps sync-wait commands per CTRL
    # instruction below what Tile's tail drain emits; split the waits across
    # separate SP nops (same engine => same ordering semantics).
    nc = self.nc
    probe = nc.sync.nop(nofuse=True)
    wait_clock.add_sem_waits(probe.ins, ScopedClock({None: tick_clock.global_clock}))
    waits = list(probe.ins.sync_info.on_wait)
    probe.ins.sync_info = bass_rust.SyncInfo(on_wait=waits[:1], on_update=[])
    for wval in waits[1:]:
        n2 = nc.sync.nop(nofuse=True)
        n2.ins.sync_info = bass_rust.SyncInfo(on_wait=[wval], on_update=[])
    nc.sync.drain()
    nc.all_engine_barrier()
    popped = nc._tile_sem_poison_stack.pop()
    assert popped is self._sem_poison
    # Inline clear_and_free_semaphores, but spread the sem clears over all
    # engines (they serialize ~30ns each; ~250 sems on one engine is ~7us of
    # tail). dma_reset must stay on gpsimd. No trailing all_engine_barrier:
    # NEFF completion waits for every engine to halt anyway, so the next
    # execution still sees cleared semaphores.
    from concourse.bass import compact_to_ranges

    sems = list(self.sems.allocated().values())
    if sems:
        sem_nums = [s.num if hasattr(s, "num") else s for s in sems]
        engines = [nc.gpsimd, nc.vector, nc.scalar, nc.tensor, nc.sync]
        for sem_range in compact_to_ranges(sem_nums):
            assert nc._state.free_isdisjoint(sem_range)
            nc.gpsimd.dma_reset(sem_range)
            n = len(sem_range)
            n_eng = len(engines)
            step = (n + n_eng - 1) // n_eng
            for ei, lo in enumerate(range(0, n, step)):
                sub = range(sem_range.start + lo, sem_range.start + min(lo + step, n))
                engines[ei % n_eng].sem_clear(sub)
        nc._state.prepend_free_semaphores(sem_nums)
        for poison_set in nc._tile_sem_poison_stack:
            poison_set.update(sem_nums)


tile.TileContext._drain_and_barrier = _patched_drain_and_barrier

_MAX_WAITS = 1  # walrus merges Ldweights+Matmult waits into one struct capped at 2


def _split_sync_waits(nc, max_waits=_MAX_WAITS):
    # Hoist sem waits beyond the per-instruction cap onto same-engine NoOps
    # inserted right before the offender (same engine => same order semantics).
    # For Matmult preceded by its Ldweights, nops go before the Ldweights so
    # walrus can still fuse the pair (their waits are summed in the MM struct).
    n_nops = 0
    for f in nc.m.functions:
        for bb in f.blocks:
            new_insts = []
            changed = False
            for inst in bb.instructions:
                si = getattr(inst, "sync_info", None)
                waits = list(si.on_wait) if si is not None else []
                if len(waits) > max_waits:
                    head, rest = waits[:-max_waits], waits[-max_waits:]
                    pos = len(new_insts)
                    if (
                        isinstance(inst, mybir.InstMatmult)
                        and new_insts
                        and isinstance(new_insts[-1], mybir.InstLdweights)
                    ):
                        pos -= 1
                    nops = []
                    for i0 in range(0, len(head), max_waits):
                        nops.append(
                            mybir.InstNoOp(
                                name=f"{inst.name}-wsplit{i0}",
                                sync_info=mybir.SyncInfo(
                                    on_wait=head[i0 : i0 + max_waits], on_update=[]
                                ),
                                bass_nofuse=True,
                                engine=inst.engine,
                            )
                        )
                        n_nops += 1
                    new_insts[pos:pos] = nops
                    inst.sync_info = mybir.SyncInfo(
                        on_wait=rest, on_update=list(si.on_update)
                    )
                    changed = True
                new_insts.append(inst)
            if changed:
                bb.instructions = new_insts
    return n_nops


def _build_fp8():
    """fp8 DoubleRow variant: contraction dims pair-packed as [128, 2, n].

    Pair layout: virtual contraction row (pair, p, i) = index pair*256 + i*128 + p.
    lhsT and rhs use the same (p, i) mapping, so the DoubleRow pairing is
    consistent regardless of the hardware's internal interleave order.
    """
    nc = bass.Bass()
    x8 = nc.dram_tensor("x8", [2, P, 2, Q], FP8E4, kind="ExternalInput")
    y8 = nc.dram_tensor("y8", [2, P, 2, K], FP8E4, kind="ExternalInput")
    A8 = nc.dram_tensor("A8", [2, P, 2, E], FP8E4, kind="ExternalInput")
    Wvo8 = nc.dram_tensor("Wvo8", [2, P, 2, E], FP8E4, kind="ExternalInput")
    out = nc.dram_tensor("out", [Q, E], F32, kind="ExternalOutput")

    exp = mybir.ActivationFunctionType.Exp
    DR = mybir.MatmulPerfMode.DoubleRow
    KP = KT // 2  # 16 k-pair tiles
    # exp shift: P' = exp(s/sqrt(512) - C) fits e4m3 (max logit ~8.1 -> P' <= 62);
    # the flushed tail (weights < 2^-9 of e^C) carries ~1e-3 of the softmax mass.
    C_SHIFT = 4.0

    with tile.TileContext(nc) as tc:
        with (
            tc.tile_pool(name="const", bufs=1) as cpool,
            tc.tile_pool(name="pwork", bufs=4) as wpool,
            tc.tile_pool(name="outp", bufs=2) as opool,
            tc.tile_pool(name="ps_mm", bufs=3, space="PSUM") as ps_mm,
            tc.tile_pool(name="ps_att", bufs=1, space="PSUM") as ps_att,
            tc.tile_pool(name="ps_den", bufs=1, space="PSUM") as ps_den,
        ):
            x8_sb = [cpool.tile([P, 2, Q], FP8E4, name=f"x8{i}") for i in range(2)]
            A8_sb = [cpool.tile([P, 2, E], FP8E4, name=f"A8{i}") for i in range(2)]
            y8_sb = [cpool.tile([P, 2, K], FP8E4, name=f"y8{i}") for i in range(2)]
            Wv8_sb = [cpool.tile([P, 2, E], FP8E4, name=f"Wv8{i}") for i in range(2)]
            t8_sb = [cpool.tile([P, 2, Q], FP8E4, name=f"t8{i}") for i in range(2)]
            Vp8_sb = [cpool.tile([P, 2, E], FP8E4, name=f"Vp8{i}") for i in range(KP)]
            ones_sb = cpool.tile([P, 32], FP8E4, name="ones")
            nc.vector.memset(ones_sb[:], 1.0)
            bias_sb = cpool.tile([P, 1], F32, name="biasC")
            nc.vector.memset(bias_sb[:], -C_SHIFT)
            # rhs AP [128, 2, 1] with middle step 16 (DoubleRow needs step%16==0)
            ones_ap = ones_sb.rearrange("p (i c) -> p i c", c=16)[:, :, 0:1]

            # Input DMAs spread across engine queues so descriptor generation
            # runs in parallel; phase-1's needs (A8 + first x8 halves) issue first.
            for i in range(2):
                nc.scalar.dma_start(A8_sb[i][:], A8[i])
            for half in range(2):
                for i in range(2):
                    nc.sync.dma_start(
                        x8_sb[i][:, :, half * 1024 : (half + 1) * 1024],
                        x8[i][:, :, half * 1024 : (half + 1) * 1024],
                    )
            for i in range(2):
                nc.gpsimd.dma_start(Wv8_sb[i][:], Wvo8[i])
            # y8 is large and only needed from phase 2 on; defer it behind
            # phase-1 progress so the xT/A loads get the full DMA bandwidth
            y8_dmas = []
            for half in range(2):
                for i in range(2):
                    y8_dmas.append(
                        nc.gpsimd.dma_start(
                            y8_sb[i][:, :, half * (K // 2) : (half + 1) * (K // 2)],
                            y8[i][:, :, half * (K // 2) : (half + 1) * (K // 2)],
                        )
                    )

            # Phase 1 (fp8 DR): tT[e2, q] = sum_e A[e, e2] * x[q, e], cast to fp8
            # pairs. qb-major so the first half of x8 unblocks 8 of 16 psums.
            p1_mms = []
            for qb in range(Q // 512):
                for e2 in range(ET):
                    pt = ps_mm.tile([P, 512], F32, name="ps_s")
                    for pr in range(2):
                        mm = nc.tensor.matmul(
                            pt[:],
                            A8_sb[pr][:, :, e2 * P : (e2 + 1) * P],
                            x8_sb[pr][:, :, qb * 512 : (qb + 1) * 512],
                            start=(pr == 0),
                            stop=(pr == 1),
                            perf_mode=DR,
                        )
                        p1_mms.append(mm)
                    nc.vector.tensor_copy(
                        t8_sb[e2 // 2][:, e2 % 2, qb * 512 : (qb + 1) * 512], pt[:]
                    )
            # release y8 loads once phase 1 is underway (xT fully consumed soon)
            for dma in y8_dmas:
                tile.add_dep_helper(
                    dma.ins, p1_mms[4].ins, sync=True, reason="defer y8 behind xT"
                )

            # Phase 2 (fp8 DR): Vp[k, f] = sum_e2 y[k, e2] WvoT[e2, f], pair-packed
            for kt in range(KT):
                pv = ps_mm.tile([P, 512], F32, name="ps_s")
                for pr in range(2):
                    nc.tensor.matmul(
                        pv[:],
                        y8_sb[pr][:, :, kt * P : (kt + 1) * P],
                        Wv8_sb[pr][:],
                        start=(pr == 0),
                        stop=(pr == 1),
                        perf_mode=DR,
                    )
                nc.scalar.copy(Vp8_sb[kt // 2][:, kt % 2, :], pv[:])

            # Phase 3: attention per 512-wide q block; att/den accumulate over k
            # pairs. Software-pipelined: S^T/exp for pair kp is emitted before
            # the att/den matmuls of pair kp-1 so the PE never waits on ACT.
            for qb in range(NQB):
                att_ps = [ps_att.tile([P, E], F32, name=f"att{j}") for j in range(NQS)]
                den_ps = ps_den.tile([P, NQS], F32, name="den")
                p8_tiles = [None] * KP
                for kp in range(KP + 1):
                    if kp < KP:
                        p8 = wpool.tile([P, 2, QB], FP8E4, name="p8")
                        p8_tiles[kp] = p8
                        for half in range(2):
                            kt = 2 * kp + half
                            st = ps_mm.tile([P, QB], F32, name="ps_s")
                            for pr in range(2):
                                nc.tensor.matmul(
                                    st[:],
                                    y8_sb[pr][:, :, kt * P : (kt + 1) * P],
                                    t8_sb[pr][:, :, qb * QB : (qb + 1) * QB],
                                    start=(pr == 0),
                                    stop=(pr == 1),
                                    perf_mode=DR,
                                )
                            nc.scalar.activation(
                                p8[:, half, :], st[:], exp, bias=bias_sb[:], scale=SCALE
                            )
                    if kp >= 1:
                        kprev = kp - 1
                        p8p = p8_tiles[kprev]
                        p8_tiles[kprev] = None
                        for j in range(NQS):
                            nc.tensor.matmul(
                                att_ps[j][:],
                                p8p[:, :, j * QS : (j + 1) * QS],
                                Vp8_sb[kprev][:],
                                start=(kprev == 0),
                                stop=(kprev == KP - 1),
                                perf_mode=DR,
                            )
                            nc.tensor.matmul(
                                den_ps[:, j : j + 1],
                                p8p[:, :, j * QS : (j + 1) * QS],
                                ones_ap,
                                start=(kprev == 0),
                                stop=(kprev == KP - 1),
                                perf_mode=DR,
                            )
                rec_sb = opool.tile([P, NQS], F32, name="rec")
                nc.vector.reciprocal(rec_sb[:], den_ps[:])
                o_sb = opool.tile([P, NQS, E], F32, name="osb")
                for j in range(NQS):
                    # alternate DVE/ACT so the last q-block's epilogue halves
                    if j % 2 == 0:
                        nc.vector.tensor_scalar_mul(
                            o_sb[:, j, :], att_ps[j][:], rec_sb[:, j : j + 1]
                        )
                    else:
                        nc.scalar.mul(o_sb[:, j, :], att_ps[j][:], rec_sb[:, j : j + 1])
                # one DMA per q-block: DRAM rows qb*512 + j*128 + p
                nc.sync.dma_start(
                    out[qb * QB : (qb + 1) * QB, :].rearrange(
                        "(j p) f -> p j f", p=P
                    ),
                    o_sb[:],
                )

    _split_sync_waits(nc)
    return nc


def _build():
    nc = bass.Bass()
    xT = nc.dram_tensor("xT", [E, Q], BF16, kind="ExternalInput")
    yT = nc.dram_tensor("yT", [E, K], BF16, kind="ExternalInput")
    A = nc.dram_tensor("A", [E, E], BF16, kind="ExternalInput")
    WvoT = nc.dram_tensor("WvoT", [E, E], BF16, kind="ExternalInput")
    out = nc.dram_tensor("out", [Q, E], F32, kind="ExternalOutput")

    exp = mybir.ActivationFunctionType.Exp

    with tile.TileContext(nc) as tc:
        with (
            tc.tile_pool(name="const", bufs=1) as cpool,
            tc.tile_pool(name="pwork", bufs=3) as wpool,
            tc.tile_pool(name="outp", bufs=4) as opool,
            tc.tile_pool(name="ps_mm", bufs=2, space="PSUM") as ps_mm,
            tc.tile_pool(name="ps_att", bufs=1, space="PSUM") as ps_att,
            tc.tile_pool(name="ps_den", bufs=2, space="PSUM") as ps_den,
        ):
            xT_sb = [cpool.tile([P, Q], BF16, name=f"xT{i}") for i in range(ET)]
            yT_sb = [cpool.tile([P, K], BF16, name=f"yT{i}") for i in range(ET)]
            A_sb = [cpool.tile([P, E], BF16, name=f"A{i}") for i in range(ET)]
            Wv_sb = [cpool.tile([P, E], BF16, name=f"Wv{i}") for i in range(ET)]
            tT_sb = [cpool.tile([P, Q], BF16, name=f"tT{i}") for i in range(ET)]
            Vp_sb = [cpool.tile([P, E], BF16, name=f"Vp{i}") for i in range(KT)]
            ones_sb = cpool.tile([P, 1], BF16, name="ones")
            nc.vector.memset(ones_sb[:], 1.0)

            for i in range(ET):
                nc.sync.dma_start(A_sb[i][:], A[i * P : (i + 1) * P, :])
                nc.sync.dma_start(xT_sb[i][:], xT[i * P : (i + 1) * P, :])
            for i in range(ET):
                nc.sync.dma_start(Wv_sb[i][:], WvoT[i * P : (i + 1) * P, :])
                nc.sync.dma_start(yT_sb[i][:], yT[i * P : (i + 1) * P, :])

            # Phase 1: tT[e2, q] = sum_e A[e, e2] * xT[e, q]
            for e2 in range(ET):
                for qb in range(Q // 512):
                    pt = ps_mm.tile([P, 512], F32, name="ps_s")
                    for et in range(ET):
                        nc.tensor.matmul(
                            pt[:],
                            A_sb[et][:, e2 * P : (e2 + 1) * P],
                            xT_sb[et][:, qb * 512 : (qb + 1) * 512],
                            start=(et == 0),
                            stop=(et == ET - 1),
                        )
                    nc.vector.tensor_copy(tT_sb[e2][:, qb * 512 : (qb + 1) * 512], pt[:])

            # Phase 2: Vp[k, f] = sum_e2 yT[e2, k] * WvoT[e2, f]
            for kt in range(KT):
                pv = ps_mm.tile([P, 512], F32, name="ps_s")
                for e2 in range(ET):
                    nc.tensor.matmul(
                        pv[:],
                        yT_sb[e2][:, kt * P : (kt + 1) * P],
                        Wv_sb[e2][:],
                        start=(e2 == 0),
                        stop=(e2 == ET - 1),
                    )
                nc.vector.tensor_copy(Vp_sb[kt][:], pv[:])

            # Phase 3: attention, one 512-wide q block at a time
            for qb in range(NQB):
                att_ps = [ps_att.tile([P, E], F32, name=f"att{j}") for j in range(NQS)]
                den_ps = ps_den.tile([P, NQS], F32, name="den")
                for kt in range(KT):
                    st = ps_mm.tile([P, QB], F32, name="ps_s")
                    for e2 in range(ET):
                        nc.tensor.matmul(
                            st[:],
                            yT_sb[e2][:, kt * P : (kt + 1) * P],
                            tT_sb[e2][:, qb * QB : (qb + 1) * QB],
                            start=(e2 == 0),
                            stop=(e2 == ET - 1),
                        )
                    p_sb = wpool.tile([P, QB], BF16, name="p_sb")
                    nc.scalar.activation(p_sb[:], st[:], exp, scale=SCALE)
                    for j in range(NQS):
                        nc.tensor.matmul(
                            att_ps[j][:],
                            p_sb[:, j * QS : (j + 1) * QS],
                            Vp_sb[kt][:],
                            start=(kt == 0),
                            stop=(kt == KT - 1),
                        )
                        nc.tensor.matmul(
                            den_ps[:, j : j + 1],
                            p_sb[:, j * QS : (j + 1) * QS],
                            ones_sb[:],
                            start=(kt == 0),
                            stop=(kt == KT - 1),
                        )
                rec_sb = opool.tile([P, NQS], F32, name="rec")
                nc.vector.reciprocal(rec_sb[:], den_ps[:])
                for j in range(NQS):
                    o_sb = opool.tile([P, E], F32, name="osb")
                    nc.vector.tensor_scalar_mul(o_sb[:], att_ps[j][:], rec_sb[:, j : j + 1])
                    nc.sync.dma_start(
                        out[qb * QB + j * QS : qb * QB + (j + 1) * QS, :], o_sb[:]
                    )

    _split_sync_waits(nc)
    return nc


_CACHED_NC = None


def _get_nc():
    global _CACHED_NC
    if _CACHED_NC is None:
        _CACHED_NC = _build_fp8() if USE_FP8 else _build()
    return _CACHED_NC


def _pair_pack(m):
    # [512, n] -> [2, 128, 2, n] with (pair, p, i) -> row pair*256 + i*128 + p
    n = m.shape[1]
    return np.ascontiguousarray(m.reshape(2, 2, P, n).transpose(0, 2, 1, 3))


def _prep_inputs(x, y, Wq, Wk, Wv, Wo):
    if USE_FP8:
        A8 = _pair_pack((Wq.T @ Wk).astype(E4_NP))
        WvoT8 = _pair_pack((Wv.T @ Wo.T).astype(E4_NP))
        x8 = np.stack([_pair_pack(x[n].T.astype(E4_NP)) for n in range(N_CORES)])
        y8 = np.stack([_pair_pack(y[n].T.astype(E4_NP)) for n in range(N_CORES)])
        return [
            {"x8": x8[n], "y8": y8[n], "A8": A8, "Wvo8": WvoT8}
            for n in range(N_CORES)
        ]
    A = (Wq.T @ Wk).astype(BF16_NP)
    xT = x.transpose(0, 2, 1).astype(BF16_NP)
    WvoT = (Wv.T @ Wo.T).astype(BF16_NP)
    yT = y.transpose(0, 2, 1).astype(BF16_NP)
    return [
        {"xT": xT[n], "yT": yT[n], "A": A, "WvoT": WvoT} for n in range(N_CORES)
    ]


def run_device(x, y, Wq, Wk, Wv, Wo, **spmd_kwargs):
    nc = _get_nc()
    in_maps = _prep_inputs(x, y, Wq, Wk, Wv, Wo)
    res = run_bass_kernel_spmd(nc, in_maps, core_ids=list(range(N_CORES)), **spmd_kwargs)
    att = np.stack([res.results[n]["out"] for n in range(N_CORES)])
    return att, res


def kernel(x, y, Wq, Wk, Wv, Wo, bo):
    x = np.asarray(x, dtype=np.float32)
    y = np.asarray(y, dtype=np.float32)
    Wq = np.asarray(Wq, dtype=np.float32)
    Wk = np.asarray(Wk, dtype=np.float32)
    Wv = np.asarray(Wv, dtype=np.float32)
    Wo = np.asarray(Wo, dtype=np.float32)
    bo = np.asarray(bo, dtype=np.float32)
    att, _ = run_device(x, y, Wq, Wk, Wv, Wo)
    return x + att + bo[None, None, :]
